# revision 9
# baseline (speedup 1.0000x reference)
"""AdaAttention distributed Bass kernel for 8 TRN2 NeuronCores.

Module (per batch b):
  xn = instancenorm(x[b]); sn = instancenorm(s[b])
  q = Wq@xn + bq; k = Wk@sn + bk; v = Wv@s[b] + bv     (1x1 convs, [C, N])
  per head h (d=64): attn = softmax(q_h^T k_h / sqrt(d)) over keys
  o_h = v_h @ attn^T;  out = Wo@o + bo + x[b]

Sharding: core i -> b = i//4, group-rank r = i%4, heads {2r, 2r+1}.
Each core: q/k/v convs for its 128 channels, attention for its 2 heads,
chunked AllGather of o (bf16) across the 4-core batch group overlapped
with attention, Wo conv (bf16) for out-channel rows [128r:128r+128],
+ residual. Host stacks the 8 [128, N] outputs.

Device layouts (per core):
  x, s       : [C=512, N=3072]  (channel-major, tokens n = t*24+j)
  wqT/wkT/wvT/woT: [512, 128]   (head/out-slice rows of W, pre-transposed)
  scores are built transposed: sT[m, n] = sum_d k[d,m] q[d,n]; softmax
  denominator = extra all-ones column in v^T (65th matmul output row);
  the divide is deferred past the attn@V matmul.
Instance norm is folded into the conv weights (scale rows by rstd,
adjust bias by -(W*rstd)@mean), so x/s are consumed raw.
"""

import numpy as np

B, C, T, J, H = 2, 512, 128, 24, 8
N = T * J                  # 3072
D = C // H                 # 64
NCORES = 8
GRPS = [[0, 1, 2, 3], [4, 5, 6, 7]]
HPC = 2                    # heads per core
CPC = HPC * D              # 128 channels per core
EPS = 1e-5
SCALE = 1.0 / float(np.sqrt(D))   # 1/8

NT = N // 512              # 6 n-chunks of 512
MT = N // 128              # 24 m-tiles of 128
CT = C // 128              # 4 channel chunks
MG = 3                     # m-tiles per exp group (3 psum banks)
NG = MT // MG              # 8 groups per (head, n-chunk) unit


def _build():
    import os

    import concourse.tile as tile
    from concourse import bacc, mybir
    from concourse.masks import make_identity

    F32 = mybir.dt.float32
    F32R = mybir.dt.float32r
    BF16 = mybir.dt.bfloat16
    DBG = os.environ.get("KERNEL_DEBUG") == "1"

    nc = bacc.Bacc("TRN2", target_bir_lowering=False, debug=False,
                   num_devices=NCORES)

    x_d = nc.dram_tensor("x", [C, N], F32, kind="ExternalInput").ap()
    s_d = nc.dram_tensor("s", [C, N], F32, kind="ExternalInput").ap()
    xres_d = nc.dram_tensor("xres", [CPC, N], F32, kind="ExternalInput").ap()
    wqT_d = nc.dram_tensor("wqT", [C, CPC], F32, kind="ExternalInput").ap()
    wkT_d = nc.dram_tensor("wkT", [C, CPC], F32, kind="ExternalInput").ap()
    wvT_d = nc.dram_tensor("wvT", [C, CPC], F32, kind="ExternalInput").ap()
    woT_d = nc.dram_tensor("woT", [C, CPC], F32, kind="ExternalInput").ap()
    bq_d = nc.dram_tensor("bq", [CPC, 1], F32, kind="ExternalInput").ap()
    bk_d = nc.dram_tensor("bk", [CPC, 1], F32, kind="ExternalInput").ap()
    bv_d = nc.dram_tensor("bv", [CPC, 1], F32, kind="ExternalInput").ap()
    bo_d = nc.dram_tensor("bo", [CPC, 1], F32, kind="ExternalInput").ap()
    out_d = nc.dram_tensor("out", [CPC, N], F32, kind="ExternalOutput").ap()
    if DBG:
        dbg_q = nc.dram_tensor("dbg_q", [CPC, N], F32, kind="ExternalOutput").ap()
        dbg_k = nc.dram_tensor("dbg_k", [CPC, N], F32, kind="ExternalOutput").ap()
        dbg_v = nc.dram_tensor("dbg_v", [CPC, N], F32, kind="ExternalOutput").ap()
        dbg_o = nc.dram_tensor("dbg_o", [CPC, N], F32, kind="ExternalOutput").ap()

    with tile.TileContext(nc) as tc:
        from contextlib import ExitStack
        with tc.tile_pool(name="persist", bufs=1) as persist, \
             tc.tile_pool(name="dram", bufs=1, space="DRAM") as dram:
            acts_scope = ExitStack()
            acts = acts_scope.enter_context(tc.tile_pool(name="acts", bufs=1))
            xt = [acts.tile([128, N], F32R, tag=f"xt{i}", name=f"xt{i}")
                  for i in range(CT)]
            st = [acts.tile([128, N], F32R, tag=f"st{i}", name=f"st{i}")
                  for i in range(CT)]
            v_sb = acts.tile([128, N], F32, tag="v_sb", name="v_sb")
            q_sb = persist.tile([128, N], F32R, tag="q_sb", name="q_sb")
            k_sb = persist.tile([128, N], F32R, tag="k_sb", name="k_sb")
            vT = [persist.tile([128, 2 * (D + 1)], F32R, tag=f"vT{m}",
                               name=f"vT{m}") for m in range(MT)]
            o_sb = persist.tile([128, N], F32, tag="o_sb", name="o_sb")
            xres = persist.tile([128, N], F32, tag="xres", name="xres")
            wq = [persist.tile([128, CPC], F32R, tag=f"wq{i}", name=f"wq{i}")
                  for i in range(CT)]
            wk = [persist.tile([128, CPC], F32R, tag=f"wk{i}", name=f"wk{i}")
                  for i in range(CT)]
            wv = [persist.tile([128, CPC], F32R, tag=f"wv{i}", name=f"wv{i}")
                  for i in range(CT)]
            wo = [persist.tile([128, CPC], BF16, tag=f"wo{i}", name=f"wo{i}")
                  for i in range(CT)]
            beff_q = persist.tile([128, 1], F32, tag="beff_q", name="beff_q")
            beff_k = persist.tile([128, 1], F32, tag="beff_k", name="beff_k")
            bv_sb = persist.tile([128, 1], F32, tag="bv_sb", name="bv_sb")
            bo_sb = persist.tile([128, 1], F32, tag="bo_sb", name="bo_sb")
            eps_sb = persist.tile([128, 1], F32, tag="eps_sb", name="eps_sb")
            ones_sb = persist.tile([128, 1], F32, tag="ones_sb", name="ones_sb")
            ident = persist.tile([128, 128], F32, tag="ident", name="ident")

            nc.vector.memset(eps_sb, EPS)
            nc.vector.memset(ones_sb, 1.0)
            make_identity(nc, ident)

            for i in range(CT):
                rows = slice(128 * i, 128 * (i + 1))
                nc.gpsimd.dma_start(out=st[i], in_=s_d[rows, :])
                nc.gpsimd.dma_start(out=xt[i], in_=x_d[rows, :])
                nc.gpsimd.dma_start(out=wv[i], in_=wvT_d[rows, :])
                nc.gpsimd.dma_start(out=wq[i], in_=wqT_d[rows, :])
                nc.gpsimd.dma_start(out=wk[i], in_=wkT_d[rows, :])
                nc.gpsimd.dma_start(out=wo[i], in_=woT_d[rows, :])
            nc.sync.dma_start(out=xres, in_=xres_d[:, :])
            nc.sync.dma_start(out=bv_sb, in_=bv_d[:, :])
            nc.sync.dma_start(out=bo_sb, in_=bo_d[:, :])

            # ---- stage 1: v conv + v^T (PE) || instance-norm stats (DVE) ----
            conv_scope = ExitStack()
            cps = conv_scope.enter_context(
                tc.tile_pool(name="conv_ps", bufs=3, space="PSUM"))
            stats_scope = ExitStack()
            stats_pool = stats_scope.enter_context(
                tc.tile_pool(name="stats", bufs=2))
            sps = stats_scope.enter_context(
                tc.tile_pool(name="stats_ps", bufs=2, space="PSUM"))

            for nj in range(NT):
                nsl = slice(512 * nj, 512 * (nj + 1))
                pv = cps.tile([128, 512], F32, tag="conv", name="conv")
                for i in range(CT):
                    nc.tensor.matmul(pv, lhsT=wv[i], rhs=st[i][:, nsl],
                                     start=(i == 0), stop=(i == CT - 1))
                nc.vector.tensor_copy(v_sb[:, nsl], pv)

            mean = {}
            for name, tiles in (("x", xt), ("s", st)):
                for i in range(CT):
                    stt = stats_pool.tile([128, NT, 6], F32, tag="bn", name="bn")
                    for j in range(NT):
                        nc.vector.bn_stats(
                            out=stt[:, j, :],
                            in_=tiles[i].bitcast(F32)[:, 512 * j:512 * (j + 1)])
                    mv = stats_pool.tile([128, 2], F32, tag=f"mv_{name}{i}",
                                         name=f"mv_{name}{i}")
                    nc.vector.bn_aggr(out=mv, in_=stt)
                    rstd = stats_pool.tile([128, 1], F32, tag=f"rstd_{name}{i}",
                                           name=f"rstd_{name}{i}")
                    nc.scalar.activation(out=rstd, in_=mv[:, 1:2],
                                         func=mybir.ActivationFunctionType.Sqrt,
                                         bias=eps_sb, scale=1.0)
                    nc.vector.reciprocal(out=rstd, in_=rstd)
                    w = wq if name == "x" else wk
                    nc.vector.tensor_scalar_mul(w[i], w[i].bitcast(F32), rstd)
                    mean[(name, i)] = mv

            # v^T tiles via PE transpose; all-ones columns 64/129 provide the
            # softmax denominator as the 65th attn@V output row
            for m in range(MT):
                msl = slice(128 * m, 128 * (m + 1))
                pt = cps.tile([128, 128], F32, tag="tr", name="tr")
                nc.tensor.transpose(pt, v_sb[:, msl], ident)
                nc.vector.tensor_copy(vT[m][:, 0:D], pt[:, 0:D])
                nc.vector.tensor_copy(vT[m][:, D + 1:2 * D + 1], pt[:, D:2 * D])
                nc.vector.tensor_copy(vT[m][:, D:D + 1], ones_sb)
                nc.vector.tensor_copy(vT[m][:, 2 * D + 1:2 * D + 2], ones_sb)

            # beff = b - (W*rstd)^T @ mean
            for name, w, b_dram, beff in (("x", wq, bq_d, beff_q),
                                          ("s", wk, bk_d, beff_k)):
                mps = sps.tile([128, 1], F32, tag="mps", name="mps")
                for i in range(CT):
                    nc.tensor.matmul(mps, lhsT=w[i].bitcast(F32),
                                     rhs=mean[(name, i)][:, 0:1],
                                     start=(i == 0), stop=(i == CT - 1))
                nc.sync.dma_start(out=beff, in_=b_dram[:, :])
                nc.vector.tensor_sub(beff, beff, mps)
            stats_scope.close()

            # ---- stage 2: q, k convs ----
            for nj in range(NT):
                nsl = slice(512 * nj, 512 * (nj + 1))
                pq = cps.tile([128, 512], F32, tag="conv", name="conv")
                for i in range(CT):
                    nc.tensor.matmul(pq, lhsT=wq[i], rhs=xt[i][:, nsl],
                                     start=(i == 0), stop=(i == CT - 1))
                nc.vector.tensor_scalar_add(q_sb[:, nsl], pq, beff_q)
                pk = cps.tile([128, 512], F32, tag="conv", name="conv")
                for i in range(CT):
                    nc.tensor.matmul(pk, lhsT=wk[i], rhs=st[i][:, nsl],
                                     start=(i == 0), stop=(i == CT - 1))
                nc.vector.tensor_scalar_add(k_sb[:, nsl], pk, beff_k)

            if DBG:
                nc.sync.dma_start(out=dbg_q, in_=q_sb.bitcast(F32))
                nc.sync.dma_start(out=dbg_k, in_=k_sb.bitcast(F32))
                nc.sync.dma_start(out=dbg_v, in_=v_sb)
            conv_scope.close()
            acts_scope.close()

            # ---- stage 3: attention + chunked AllGather + Wo conv, fused ----
            with tc.tile_pool(name="sT", bufs=2, space="PSUM") as sT_pool, \
                 tc.tile_pool(name="oacc", bufs=1, space="PSUM") as oacc_pool, \
                 tc.tile_pool(name="out_ps", bufs=1, space="PSUM") as ops, \
                 tc.tile_pool(name="eT", bufs=10) as eT_pool, \
                 tc.tile_pool(name="of", bufs=2) as ofp, \
                 tc.tile_pool(name="out_sb", bufs=3) as osb, \
                 tc.tile_pool(name="attn_sm", bufs=4) as sm_pool:
                for nj in range(NT):
                    nsl = slice(512 * nj, 512 * (nj + 1))
                    for h in range(HPC):
                        hsl = slice(D * h, D * (h + 1))
                        vsl = slice((D + 1) * h, (D + 1) * (h + 1))
                        oacc = oacc_pool.tile([D + 1, 512], F32, tag="oacc",
                                              name="oacc")
                        eTs = []

                        def av_group(g):
                            for u in range(MG):
                                m = g * MG + u
                                nc.tensor.matmul(oacc, lhsT=vT[m][:, vsl],
                                                 rhs=eTs[g][:, u, :],
                                                 start=(m == 0),
                                                 stop=(m == MT - 1))

                        for g in range(NG):
                            sT = sT_pool.tile([128, MG, 512], F32, tag="sT",
                                              name="sT")
                            for u in range(MG):
                                m = g * MG + u
                                msl = slice(128 * m, 128 * (m + 1))
                                nc.tensor.matmul(sT[:, u, :],
                                                 lhsT=k_sb[hsl, msl],
                                                 rhs=q_sb[hsl, nsl],
                                                 start=True, stop=True)
                            eT = eT_pool.tile([128, MG, 512], F32R, tag="eT",
                                              name="eT")
                            nc.scalar.activation(
                                out=eT, in_=sT,
                                func=mybir.ActivationFunctionType.Exp,
                                scale=SCALE)
                            eTs.append(eT)
                            if g >= 1:
                                av_group(g - 1)
                        av_group(NG - 1)

                        # o = o_unnorm * (1/colsum) + bv  (deferred softmax div)
                        cs_sb = sm_pool.tile([1, 512], F32, tag="cs_sb",
                                             name="cs_sb")
                        nc.vector.tensor_copy(cs_sb, oacc[D:D + 1, :])
                        recip = sm_pool.tile([1, 512], F32, tag="recip",
                                             name="recip")
                        nc.vector.reciprocal_approx_fast(recip, cs_sb)
                        rb = sm_pool.tile([D, 512], F32, tag="rb", name="rb")
                        nc.gpsimd.partition_broadcast(rb, recip)
                        nc.vector.tensor_mul(o_sb[hsl, nsl], oacc[0:D, :], rb)
                        nc.vector.tensor_scalar_add(o_sb[hsl, nsl],
                                                    o_sb[hsl, nsl],
                                                    bv_sb[hsl, :])

                    # chunked AllGather (bf16) + Wo conv for this n-slice
                    ag_in = dram.tile([CPC, 512], BF16, tag=f"ag_in{nj}",
                                      name=f"ag_in{nj}")
                    ag_out = dram.tile([C, 512], BF16, tag=f"ag_out{nj}",
                                       name=f"ag_out{nj}")
                    nc.gpsimd.dma_start(out=ag_in, in_=o_sb[:, nsl])
                    nc.gpsimd.collective_compute(
                        "AllGather", mybir.AluOpType.bypass,
                        replica_groups=GRPS,
                        ins=[ag_in[:].opt()], outs=[ag_out[:].opt()])
                    of = [ofp.tile([128, 512], BF16, tag=f"of{i}",
                                   name=f"of{i}") for i in range(CT)]
                    for i in range(CT):
                        nc.sync.dma_start(out=of[i],
                                          in_=ag_out[128 * i:128 * (i + 1), :])
                    po = ops.tile([128, 512], F32, tag="out", name="out")
                    for i in range(CT):
                        nc.tensor.matmul(po, lhsT=wo[i], rhs=of[i],
                                         start=(i == 0), stop=(i == CT - 1))
                    ot = osb.tile([128, 512], F32, tag="ot", name="ot")
                    nc.vector.scalar_tensor_tensor(
                        out=ot, in0=po, scalar=bo_sb, in1=xres[:, nsl],
                        op0=mybir.AluOpType.add, op1=mybir.AluOpType.add)
                    nc.sync.dma_start(out=out_d[:, nsl], in_=ot)

                if DBG:
                    nc.sync.dma_start(out=dbg_o, in_=o_sb)

    nc.compile()
    return nc


def _shard_inputs(x, s_sty, Wq_w, Wq_b, Wk_w, Wk_b, Wv_w, Wv_b, Wo_w, Wo_b):
    in_maps = []
    xf = x.reshape(B, C, N)
    sf = s_sty.reshape(B, C, N)
    for core in range(NCORES):
        b, gr = divmod(core, 4)
        ch = slice(CPC * gr, CPC * (gr + 1))
        in_maps.append({
            "x": np.ascontiguousarray(xf[b]),
            "s": np.ascontiguousarray(sf[b]),
            "xres": np.ascontiguousarray(xf[b, ch]),
            "wqT": np.ascontiguousarray(Wq_w[ch].T),
            "wkT": np.ascontiguousarray(Wk_w[ch].T),
            "wvT": np.ascontiguousarray(Wv_w[ch].T),
            "woT": np.ascontiguousarray(Wo_w[ch].T),
            "bq": np.ascontiguousarray(Wq_b[ch, None]),
            "bk": np.ascontiguousarray(Wk_b[ch, None]),
            "bv": np.ascontiguousarray(Wv_b[ch, None]),
            "bo": np.ascontiguousarray(Wo_b[ch, None]),
        })
    return in_maps


_NC_CACHE = {}


def _get_nc():
    if "nc" not in _NC_CACHE:
        _NC_CACHE["nc"] = _build()
    return _NC_CACHE["nc"]


def run(inputs, trace=False, **kw):
    from concourse import bass_utils
    nc = _get_nc()
    in_maps = _shard_inputs(**inputs)
    res = bass_utils.run_bass_kernel_spmd(
        nc, in_maps, core_ids=list(range(NCORES)), trace=trace, **kw)
    outs = [np.asarray(res.results[i]["out"]) for i in range(NCORES)]
    full = np.empty((B, C, T, J), np.float32)
    for core in range(NCORES):
        b, gr = divmod(core, 4)
        full[b, CPC * gr:CPC * (gr + 1)] = outs[core].reshape(CPC, T, J)
    return full, res


def kernel(**inputs):
    full, _ = run(inputs, trace=False)
    return full
